# revision 2
# baseline (speedup 1.0000x reference)
"""Multi-head attention (B=4, N=2048, C=256, H=8) on 8 Trainium2 NeuronCores.

Sharding: core c handles batch b = c//2 and query-half qh = c%2 (1024 query
rows), all 8 heads. k/v are computed for the full sequence on each core (the
qkv projection is cheap); outputs concatenate with no cross-core reduction.

Device-side layout is fully "transposed" (channels on partitions):
  - x^T [C, N] feeds q^T/k^T ([d, tokens], head-major rows) and v ([tokens, d]).
  - Scores are computed as S^T [k-tokens, q-tokens] so that softmax's exp
    output E^T feeds the AV matmul directly (contraction over k on partitions).
  - Softmax denominators come for free as a 33rd "ones" column appended to v.
  - O^T [channels, q] feeds the output projection directly.

v2: no PSUM->SBUF score staging. The softmax exp is split across BOTH
ScalarE and VectorE, reading scores straight from PSUM:
  - ScalarE: ACT Exp (exact), writing E^T in bf16.
  - VectorE: one-op Schraudolph exp2 bit-trick - tensor_scalar computes
    s*(scale*log2e*128) + (127*128 - C) and the int16 convert-on-write IS
    the bf16 exponent/mantissa construction (~3% rel err; softmax tolerance
    is 2e-2). Per chunk, head-even goes to ACT and head-odd to DVE; every
    4th chunk ACT takes both halves (balances ACT 1.2GHz vs DVE 0.96GHz).
E^T and v are bf16 (full PE rate at any free size); everything else f32r.
AV matmuls trail their chunk by two ticks so the PE never waits on exp.
Input rounding copies (f32 -> f32r) run on the otherwise-idle GPSIMD.
qkv projection is spread across the first iterations' chunk ticks
(worklist); norm + output projection of a finished q-block are deferred
a few chunks into the following iteration.
"""

import os
from collections import deque
from contextlib import ExitStack

import numpy as np

import concourse.bacc as bacc
import concourse.bass as bass
import concourse.mybir as mybir
import concourse.tile as tile
from concourse.bass_utils import run_bass_kernel_spmd

B, N, C = 4, 2048, 256
H, D = 8, 32
P = 128
QH = N // 2              # query rows per core
SCALE = float(D) ** -0.5
NCORES = 8
NCH = N // P             # 16 k-chunks

F32 = mybir.dt.float32
F32R = mybir.dt.float32r
BF16 = mybir.dt.bfloat16
I16 = mybir.dt.int16
EXP = mybir.ActivationFunctionType.Exp
MULT = mybir.AluOpType.mult
ADD = mybir.AluOpType.add

# Schraudolph bit-trick constants (bf16 domain, int16 convert-on-write).
LOG2E = 1.4426950408889634
A16 = SCALE * LOG2E * 128.0
B16 = 127.0 * 128.0 - 366393.0 / 65536.0

# Timing amplification for the local harness (repeat attention+proj body).
REPS = int(os.environ.get("BASS_ATTN_REPS", "1"))
REPS_MODE = os.environ.get("BASS_ATTN_REPS_MODE", "loop")  # "loop" | "unroll"
# Debug bisect: "allact" = no DVE bit-trick (ACT does every chunk fully)
VARIANT = os.environ.get("BASS_ATTN_VARIANT", "")


def _emit(tc, xT, xTq, wall, pb, y):
    nc = tc.nc
    with ExitStack() as ctx:
        ctx.enter_context(
            nc.allow_low_precision(
                reason="bf16 E^T/v for AV (softmax tolerance 2e-2)"
            )
        )
        singles = ctx.enter_context(tc.tile_pool(name="singles", bufs=1))
        epool = ctx.enter_context(tc.tile_pool(name="epool", bufs=4))
        small = ctx.enter_context(tc.tile_pool(name="small", bufs=4))
        ypool = ctx.enter_context(tc.tile_pool(name="ypool", bufs=3))
        # PSUM budget (8 banks): sA 2x1 + sB 3x1 + po 2x1 + bcp 1x1
        psA = ctx.enter_context(tc.tile_pool(name="psA", bufs=2, space="PSUM"))
        psB = ctx.enter_context(tc.tile_pool(name="psB", bufs=3, space="PSUM"))
        po = ctx.enter_context(tc.tile_pool(name="po", bufs=2, space="PSUM"))
        bcp = ctx.enter_context(tc.tile_pool(name="bcp", bufs=1, space="PSUM"))

        # ---- input loads + fp32r rounding ------------------------------
        # all four weight matrices arrive packed in one dram tensor (one DMA);
        # order: wk, wq, wv, pw
        wall_ld = singles.tile([P, 2, 4 * C], F32, tag="wall_ld", name="wall_ld")
        # separate tiles per weight so staged rounding copies on different
        # engines don't chain through a single-tile WAW hazard
        wk_t = singles.tile([P, 2, C], F32R, tag="wk", name="wk_t")
        wq_t = singles.tile([P, 2, C], F32R, tag="wq", name="wq_t")
        wv_t = singles.tile([P, 2, C], F32R, tag="wv", name="wv_t")
        pw_t = singles.tile([P, 2, C], F32R, tag="pw", name="pw_t")
        wk_sb = wk_t[:]
        wq_sb = wq_t[:]
        wv_sb = wv_t[:]
        pw_sb = pw_t[:]
        # x^T loads staged + rounded in 512-column blocks so the first qkv
        # matmuls (and the attention stream behind them) start early.
        xT_ld = singles.tile([P, 2, N], F32, tag="xT_ld", name="xT_ld")
        xT_sb = singles.tile([P, 2, N], F32R, tag="xT", name="xT_sb")
        xT_r = xT.rearrange("(c p) n -> p c n", p=P)

        def load_x_block(nb, eng):
            sl = (slice(None), slice(None), slice(512 * nb, 512 * nb + 512))
            nc.sync.dma_start(xT_ld[sl], xT_r[sl])
            eng.tensor_copy(xT_sb[sl], xT_ld[sl])

        wall_r = wall.rearrange("(c p) n -> p c n", p=P)
        load_x_block(0, nc.vector)
        # wk+wq slice first: it gates the first matmuls
        wsl = (slice(None), slice(None), slice(0, 2 * C))
        nc.sync.dma_start(wall_ld[wsl], wall_r[wsl])
        nc.vector.tensor_copy(wk_sb, wall_ld[:, :, 0:C])
        nc.scalar.copy(wq_sb, wall_ld[:, :, C : 2 * C])
        xTq_ld = singles.tile([P, 2, QH], F32, tag="xTq_ld", name="xTq_ld")
        xTq_sb = singles.tile([P, 2, QH], F32R, tag="xTq", name="xTq_sb")
        xTq_r = xTq.rearrange("(c p) n -> p c n", p=P)

        def load_xq_block(nb, eng):
            sl = (slice(None), slice(None), slice(512 * nb, 512 * nb + 512))
            nc.sync.dma_start(xTq_ld[sl], xTq_r[sl])
            eng.tensor_copy(xTq_sb[sl], xTq_ld[sl])

        load_xq_block(0, nc.vector)
        wsl2 = (slice(None), slice(None), slice(2 * C, 4 * C))
        nc.sync.dma_start(wall_ld[wsl2], wall_r[wsl2])
        nc.gpsimd.tensor_copy(wv_sb, wall_ld[:, :, 2 * C : 3 * C])
        nc.gpsimd.tensor_copy(pw_sb, wall_ld[:, :, 3 * C : 4 * C])
        for nb in range(1, 4):
            load_x_block(nb, nc.gpsimd)
        load_xq_block(1, nc.gpsimd)
        # pb as per-partition column for the transposed projection's ACT bias
        # add: pbT[p, j] = pb[128j + p]
        pbT_sb = singles.tile([P, 2], F32, tag="pbT")
        nc.sync.dma_start(
            pbT_sb[:],
            bass.AP(tensor=pb.tensor, offset=pb.offset, ap=[[1, P], [P, 2]]),
        )

        # ---- qkv projection emitters ------------------------------------
        # q^T/k^T stacks: chunk cc holds heads 4cc..4cc+3 at rows 32*(h%4).
        qT_sb = singles.tile([P, 2, QH], F32R, tag="qT")
        kT_sb = singles.tile([P, 2, N], F32R, tag="kT")
        # v_aug: [token-tile, head-major (v_h | 1)] bf16 for AV + denominator.
        vA_sb = singles.tile([P, NCH, H * (D + 1)], BF16, tag="vA")
        onesF = singles.tile([P, NCH, H], F32, tag="onesF")
        nc.vector.memset(onesF[:], 1.0)
        vA4 = vA_sb[:].rearrange("p t (h a) -> p t h a", a=D + 1)
        nc.vector.tensor_copy(vA4[:, :, :, D], onesF[:])

        def emit_kqT(w_sb, x_sb, out_sb, cc, nb, eng):
            pk = bcp.tile([P, 512], F32, tag="bc", name="pk")
            for ci in range(2):
                nc.tensor.matmul(
                    pk[:],
                    lhsT=w_sb[:, ci, 128 * cc : 128 * cc + 128],
                    rhs=x_sb[:, ci, 512 * nb : 512 * nb + 512],
                    start=(ci == 0),
                    stop=(ci == 1),
                )
            eng(out_sb[:, cc, 512 * nb : 512 * nb + 512], pk[:])

        def emit_v(tt, eng):
            pv = bcp.tile([P, 512], F32, tag="bc", name="pv")
            for ci in range(2):
                nc.tensor.matmul(
                    pv[:, 0:256],
                    lhsT=xT_sb[:, ci, 128 * tt : 128 * tt + 128],
                    rhs=wv_sb[:, ci, :],
                    start=(ci == 0),
                    stop=(ci == 1),
                )
            eng(
                vA4[:, tt, :, 0:D],
                pv[:, 0:256].rearrange("p (h d) -> p h d", d=D),
            )

        dve_cp = nc.vector.tensor_copy
        act_cp = nc.scalar.copy

        # ---- attention helpers ------------------------------------------
        ones_bf = singles.tile([1, 32], BF16, tag="ones")
        nc.vector.memset(ones_bf[:], 1.0)
        OT_sb = singles.tile([P, 2, QH], F32R, tag="OT")

        IDENT = mybir.ActivationFunctionType.Identity

        def emit_proj_half(qb, j):
            # transposed projection: yT[128j:128j+128, 512qb:512qb+512] =
            # pw[:, 128j:...].T @ O^T[:, 512qb:...] ; bias add on ACT with a
            # per-partition scalar (pb column)
            py = bcp.tile([P, 512], F32, tag="bc", name="py")
            for ci in range(2):
                nc.tensor.matmul(
                    py[:],
                    lhsT=pw_sb[:, ci, 128 * j : 128 * j + 128],
                    rhs=OT_sb[:, ci, 512 * qb : 512 * qb + 512],
                    start=(ci == 0),
                    stop=(ci == 1),
                )
            ysb = ypool.tile([P, 512], F32, tag="y", name="ysb")
            nc.scalar.activation(ysb[:], py[:], IDENT, bias=pbT_sb[:, j : j + 1])
            nc.sync.dma_start(
                y[128 * j : 128 * j + 128, 512 * qb : 512 * qb + 512], ysb[:]
            )

        def emit_norm_read(pot):
            # part 1: read the accumulator out of PSUM (frees the po slot):
            # 1/den on DVE, numerator copy on ACT
            rcp = small.tile([1, 512], BF16, tag="rcp", name="rcp")
            nc.vector.reciprocal(rcp[:], pot[D : D + 1, :])
            onr = small.tile([32, 512], F32, tag="onr", name="onr")
            nc.scalar.copy(onr[:], pot[0:D, :])
            return rcp, onr

        def emit_norm_mul(rcp, onr, hp, qb, e):
            # part 2: broadcast 1/den (PE) and scale the numerator rows into
            # O^T (DVE)
            h = 2 * hp + e
            r, cc = 32 * (h % 4), h // 4
            bc = bcp.tile([32, 512], F32, tag="bc", name="bc")
            nc.tensor.matmul(
                bc[:], lhsT=ones_bf[:], rhs=rcp[:], start=True, stop=True
            )
            nc.vector.tensor_mul(
                OT_sb[r : r + 32, cc, 512 * qb : 512 * qb + 512],
                onr[:],
                bc[:],
            )

        def emit_norm_head(pot, hp, qb, e):
            emit_norm_mul(*emit_norm_read(pot), hp, qb, e)

        def emit_av_one(pots, hp, etA, etD, ch):
            # etA (ACT) and etD (DVE) each hold one head's E^T for chunk ch
            st, sp = (ch == 0), (ch == NCH - 1)
            nc.tensor.matmul(
                pots[0][:], lhsT=vA4[:, ch, 2 * hp], rhs=etA[:],
                start=st, stop=sp, skip_group_check=True,
            )
            nc.tensor.matmul(
                pots[1][:], lhsT=vA4[:, ch, 2 * hp + 1], rhs=etD[:],
                start=st, stop=sp, skip_group_check=True,
            )

        def emit_body():
            # ---- prefix: just enough qkv for iteration 0's first chunks ----
            emit_kqT(wk_sb, xT_sb, kT_sb, 0, 0, dve_cp)
            emit_kqT(wq_sb, xTq_sb, qT_sb, 0, 0, act_cp)
            for tt in range(4):
                emit_v(tt, (act_cp, dve_cp)[tt % 2])
            # remaining qkv work, spread one item per chunk tick; deadlines:
            # v_tt by tick tt, kT(0,nb) before tick 4nb, cc=1 before tick 32.
            worklist = [
                lambda: emit_v(4, act_cp),
                lambda: emit_kqT(wk_sb, xT_sb, kT_sb, 0, 1, dve_cp),
                lambda: emit_v(5, act_cp),
                lambda: emit_v(6, dve_cp),
                lambda: emit_v(7, act_cp),
                lambda: emit_kqT(wk_sb, xT_sb, kT_sb, 0, 2, dve_cp),
                lambda: emit_v(8, act_cp),
                lambda: emit_v(9, dve_cp),
                lambda: emit_v(10, act_cp),
                lambda: emit_kqT(wk_sb, xT_sb, kT_sb, 0, 3, dve_cp),
                lambda: emit_v(11, act_cp),
                lambda: emit_v(12, dve_cp),
                lambda: emit_v(13, act_cp),
                lambda: emit_v(14, dve_cp),
                lambda: emit_v(15, act_cp),
                lambda: emit_kqT(wq_sb, xTq_sb, qT_sb, 0, 1, dve_cp),
                lambda: emit_kqT(wk_sb, xT_sb, kT_sb, 1, 0, act_cp),
                lambda: emit_kqT(wk_sb, xT_sb, kT_sb, 1, 1, dve_cp),
                lambda: emit_kqT(wk_sb, xT_sb, kT_sb, 1, 2, act_cp),
                lambda: emit_kqT(wk_sb, xT_sb, kT_sb, 1, 3, dve_cp),
                lambda: emit_kqT(wq_sb, xTq_sb, qT_sb, 1, 0, act_cp),
                lambda: emit_kqT(wq_sb, xTq_sb, qT_sb, 1, 1, dve_cp),
            ]

            # ---- attention main loop ---------------------------------------
            av_q = deque()      # (pots, hp, et, ch): AV trails by two ticks
            deferred = []       # norm/proj actions, one per tick
            body_reps = REPS if (REPS > 1 and REPS_MODE == "unroll") else 1
            its = [
                (qb, hp)
                for _ in range(body_reps)
                for qb in range(QH // 512)
                for hp in range(H // 2)
            ]
            pots = None
            for qb, hp in its:
                for ch in range(NCH):
                    # one PSUM tile per head half so the two exp engines are
                    # independent readers (same-tile readers get chained by
                    # the dependency tracker, serializing ACT->DVE)
                    sA = psA.tile([P, 512], F32, tag="sA", name="sA")
                    sB = psB.tile([P, 512], F32, tag="sB", name="sB")
                    for e, st in ((0, sA), (1, sB)):
                        h = 2 * hp + e
                        r, cc = 32 * (h % 4), h // 4
                        nc.tensor.matmul(
                            st[:],
                            lhsT=kT_sb[r : r + 32, cc, 128 * ch : 128 * ch + 128],
                            rhs=qT_sb[r : r + 32, cc, 512 * qb : 512 * qb + 512],
                            start=True,
                            stop=True,
                            tile_position=(r, 0),
                        )
                    etA = epool.tile([P, 512], BF16, tag="EA", name="etA")
                    etD = epool.tile([P, 512], BF16, tag="ED", name="etD")
                    nc.scalar.activation(etA[:], sA[:], EXP, scale=SCALE)
                    nc.vector.tensor_scalar(
                        etD[:].bitcast(I16), sB[:], A16, B16, MULT, ADD
                    )
                    if ch == 0:
                        pots = (
                            po.tile([D + 1, 512], F32, tag="o", name="pot0"),
                            po.tile([D + 1, 512], F32, tag="o", name="pot1"),
                        )
                    av_q.append((pots, hp, etA, etD, ch))
                    if ch == NCH - 1:
                        # drain: all AVs must be emitted before the norms of
                        # this iteration's pots (deferred to next iteration's
                        # first ticks) are emitted
                        while av_q:
                            emit_av_one(*av_q.popleft())
                    elif len(av_q) > 3:
                        emit_av_one(*av_q.popleft())
                    if deferred:
                        deferred.pop(0)()
                    elif worklist:
                        worklist.pop(0)()
                if (qb, hp) != its[-1]:
                    # ticks 0-1 of the next iteration: free the accumulators;
                    # ticks 2-3: finish the normalization; then projection
                    nrm = {}

                    def _read(pt, e, nrm=nrm):
                        nrm[e] = emit_norm_read(pt)

                    def _mul(e, b, c, nrm=nrm):
                        emit_norm_mul(*nrm[e], b, c, e)

                    deferred.append(lambda a=pots[0]: _read(a, 0))
                    deferred.append(lambda a=pots[1]: _read(a, 1))
                    deferred.append(lambda b=hp, c=qb: _mul(0, b, c))
                    deferred.append(lambda b=hp, c=qb: _mul(1, b, c))
                    if hp == H // 2 - 1:
                        for j in range(2):
                            deferred.append(lambda a=qb, b=j: emit_proj_half(a, b))
            while av_q:
                emit_av_one(*av_q.popleft())
            if "dump" in VARIANT:
                nc_ = tc.nc
                p0d = nc_.dram_tensor(
                    "p0d", [D + 1, 512], F32, kind="ExternalOutput"
                ).ap()
                p1d = nc_.dram_tensor(
                    "p1d", [D + 1, 512], F32, kind="ExternalOutput"
                ).ap()
                pc0 = singles.tile([D + 1, 512], F32, tag="pc0")
                nc.vector.tensor_copy(pc0[:], pots[0][:])
                nc_.sync.dma_start(p0d, pc0[:])
                pc1 = singles.tile([D + 1, 512], F32, tag="pc1")
                nc.vector.tensor_copy(pc1[:], pots[1][:])
                nc_.sync.dma_start(p1d, pc1[:])
            for act in deferred:
                act()
            l_qb, l_hp = its[-1]
            n0 = emit_norm_read(pots[0])
            n1 = emit_norm_read(pots[1])
            emit_norm_mul(*n0, l_hp, l_qb, 0)
            emit_norm_mul(*n1, l_hp, l_qb, 1)
            for j in range(2):
                emit_proj_half(l_qb, j)

        if REPS == 1 or REPS_MODE == "unroll":
            emit_body()
        else:
            with tc.For_i(0, REPS, 1):
                emit_body()

        if "dump" in VARIANT:
            nc_ = tc.nc
            kTd = nc_.dram_tensor("kTd", [P, 2, N], F32, kind="ExternalOutput").ap()
            qTd = nc_.dram_tensor("qTd", [P, 2, QH], F32, kind="ExternalOutput").ap()
            vAd = nc_.dram_tensor(
                "vAd", [P, NCH, H * (D + 1)], BF16, kind="ExternalOutput"
            ).ap()
            OTd = nc_.dram_tensor("OTd", [P, 2, QH], F32, kind="ExternalOutput").ap()
            nc_.sync.dma_start(kTd, kT_sb[:].bitcast(F32))
            nc_.sync.dma_start(qTd, qT_sb[:].bitcast(F32))
            nc_.sync.dma_start(vAd, vA_sb[:])
            nc_.sync.dma_start(OTd, OT_sb[:].bitcast(F32))


_NC = None
_RUNNER = None


def _get_runner():
    """Cached SPMD runner: builds the jitted shard_map executable once so warm
    kernel() calls skip JAX retracing/compilation (run_bass_kernel_spmd builds
    a fresh closure per call, which always misses the jit cache)."""
    global _RUNNER
    if _RUNNER is not None:
        return _RUNNER
    import jax
    from jax.sharding import Mesh, PartitionSpec
    from jax.experimental.shard_map import shard_map
    from concourse import bass2jax, mybir as _mb

    nc = _get_nc()
    bass2jax.install_neuronx_cc_hook()

    assert nc.dbg_addr is None
    partition_name = nc.partition_id_tensor.name if nc.partition_id_tensor else None
    in_names, out_names, out_avals = [], [], []
    for alloc in nc.m.functions[0].allocations:
        if not isinstance(alloc, _mb.MemoryLocationSet):
            continue
        name = alloc.memorylocations[0].name
        if alloc.kind == "ExternalInput":
            if name != partition_name:
                in_names.append(name)
        elif alloc.kind == "ExternalOutput":
            out_names.append(name)
            out_avals.append(
                jax.core.ShapedArray(tuple(alloc.tensor_shape), _mb.dt.np(alloc.dtype))
            )
    n_params = len(in_names)
    n_outs = len(out_avals)
    all_names = in_names + out_names
    if partition_name is not None:
        all_names = all_names + [partition_name]

    def _body(*args):
        operands = list(args)
        if partition_name is not None:
            operands.append(bass2jax.partition_id_tensor())
        outs = bass2jax._bass_exec_p.bind(
            *operands,
            out_avals=tuple(out_avals),
            in_names=tuple(all_names),
            out_names=tuple(out_names),
            lowering_input_output_aliases=(),
            sim_require_finite=True,
            sim_require_nnan=True,
            nc=nc,
        )
        return tuple(outs)

    devices = jax.devices()[:NCORES]
    mesh = Mesh(np.asarray(devices), ("core",))
    sharded = jax.jit(
        shard_map(
            _body,
            mesh=mesh,
            in_specs=(PartitionSpec("core"),) * (n_params + n_outs),
            out_specs=(PartitionSpec("core"),) * n_outs,
            check_rep=False,
        ),
        donate_argnums=tuple(range(n_params, n_params + n_outs)),
        keep_unused=True,
    )

    def run(in_maps):
        concat_in = [
            np.concatenate([np.asarray(m[nm]) for m in in_maps], axis=0)
            for nm in in_names
        ]
        concat_zeros = [
            np.zeros((NCORES * a.shape[0], *a.shape[1:]), a.dtype) for a in out_avals
        ]
        out_arrs = sharded(*concat_in, *concat_zeros)
        return [
            {
                nm: np.asarray(out_arrs[i]).reshape(NCORES, *out_avals[i].shape)[c]
                for i, nm in enumerate(out_names)
            }
            for c in range(NCORES)
        ]

    _RUNNER = run
    return run


def _get_nc():
    global _NC
    if _NC is None:
        nc = bacc.Bacc("TRN2", target_bir_lowering=False, debug=False, num_devices=1)
        xT = nc.dram_tensor("xT", [C, N], F32, kind="ExternalInput").ap()
        xTq = nc.dram_tensor("xTq", [C, QH], F32, kind="ExternalInput").ap()
        wall = nc.dram_tensor("wall", [C, 4 * C], F32, kind="ExternalInput").ap()
        pb = nc.dram_tensor("pb", [C], F32, kind="ExternalInput").ap()
        y = nc.dram_tensor("y", [C, QH], F32, kind="ExternalOutput").ap()
        with tile.TileContext(nc) as tc:
            _emit(tc, xT, xTq, wall, pb, y)
        nc.finalize()
        _NC = nc
    return _NC


def kernel(x, qkv_w, proj_w, proj_b):
    x = np.asarray(x, dtype=np.float32)
    qkv_w = np.asarray(qkv_w, dtype=np.float32)
    proj_w = np.asarray(proj_w, dtype=np.float32)
    proj_b = np.asarray(proj_b, dtype=np.float32)

    nc = _get_nc()
    wall = np.ascontiguousarray(
        np.stack(
            [qkv_w[C : 2 * C].T, qkv_w[0:C].T, qkv_w[2 * C : 3 * C].T, proj_w.T],
            axis=1,
        ).reshape(C, 4 * C)
    )

    in_maps = []
    for c in range(NCORES):
        b, qh = c // 2, c % 2
        xTc = np.ascontiguousarray(x[b].T)
        in_maps.append(
            {
                "xT": xTc,
                "xTq": np.ascontiguousarray(xTc[:, qh * QH : (qh + 1) * QH]),
                "wall": wall,
                "pb": proj_b,
            }
        )
    results = _get_runner()(in_maps)
    out = np.empty((B, N, C), np.float32)
    for c in range(NCORES):
        b, qh = c // 2, c % 2
        out[b, qh * QH : (qh + 1) * QH] = results[c]["y"].T
    return out
